# revision 1
# baseline (speedup 1.0000x reference)
"""Bass/Trainium2 kernel for nn_Decoder (2-layer bidir-style LSTM decoder
with general attention + fc), distributed over 8 NeuronCores.

Architecture (SPMD, one uniform program; per-core behavior differs only in
input DATA):
  - 4 LSTM cells (L0f, L0b, L1f, L1b) -> cores 0..3; cores 4..7 mirror 0..3.
  - Sequential scan is chunked: CH=32 timesteps per chunk. One AllGather per
    chunk exchanges each cell's h-chunk. Layer 1 runs one chunk behind
    layer 0 (its input-to-hidden term consumes L0's h from the previous AG).
  - Per-chunk input-to-hidden precompute: ih = W_emb @ emb_chunk +
    W_in @ [h0f; h0b]_chunk (+bias); per-core zero weights make the same
    program compute the right thing for every cell.
  - Per step: gates = ih[:, t] + Whh @ h_{t-1} via PE (weights stationary,
    gates on partitions, batch=16 moving), then an 8-op ACT/DVE chain for
    the LSTM nonlinearity. Gate order is permuted to i,f,o,g on host so one
    Sigmoid covers i,f,o and one Tanh covers g.
  - dec_t = h1f + h1b accumulated from AG slots 2,3 into per-core DRAM.
  - Post phase (after scan): per-core, its 2 batch elements: enc_proj,
    masked softmax attention, context, fc, all as dense bf16 matmuls.

Numerics: bf16 weights/activations with fp32 PSUM accumulation and fp32
cell state c. Expected rel err ~1e-3..1e-2 vs the fp32 reference.
"""

import os
import sys

sys.path.insert(0, "/opt/trn_rl_repo")

import numpy as np
import ml_dtypes

import concourse.bass as bass
import concourse.mybir as mybir
import concourse.tile as tile
from concourse import bacc
from concourse.bass_utils import run_bass_kernel_spmd

# ---- problem constants (hardcoded per contract) ----
L = 2
H = 512
E = 512
B = 16
T = 512
S = 512
VOCAB = 1001
OUT = 1000

N_CORES = 8
CH = 32                       # timesteps per chunk
NCH = T // CH                 # 16 chunks
ITERS = NCH + 1               # L1 lags one chunk
COLS = CH * B                 # 512 columns per chunk (s-major, b-minor)
HC = H // 128                 # 4 H-chunks
MC = (4 * H) // 128           # 16 gate M-chunks
TC = T // 128                 # 4 T-chunks (post phase)
SC = S // 128                 # 4 S-chunks
BF = mybir.dt.bfloat16
F32 = mybir.dt.float32
I32 = mybir.dt.int32

# gate permutation: torch order i,f,g,o -> i,f,o,g  (rows of the 4H dim)
def _gate_perm():
    idx = np.arange(4 * H)
    return np.concatenate([idx[0:H], idx[H:2 * H], idx[3 * H:4 * H], idx[2 * H:3 * H]])


def _bf(x):
    return np.ascontiguousarray(np.asarray(x, dtype=np.float32)).astype(ml_dtypes.bfloat16)


def _f32(x):
    return np.ascontiguousarray(np.asarray(x, dtype=np.float32))


def build_nc(nch=NCH):
    iters = nch + 1
    t_total = nch * CH
    tc_n = t_total // 128
    nc = bacc.Bacc("TRN2", target_bir_lowering=False, debug=False, num_devices=N_CORES)

    # ---- DRAM inputs ----
    w_emb = nc.dram_tensor("w_emb", [E, 4 * H], BF, kind="ExternalInput")
    w_in = nc.dram_tensor("w_in", [2 * H, 4 * H], BF, kind="ExternalInput")
    w_hh = nc.dram_tensor("w_hh", [H, 4 * H], BF, kind="ExternalInput")
    biasp = nc.dram_tensor("biasp", [128, MC], F32, kind="ExternalInput")
    h_init = nc.dram_tensor("h_init", [128, HC, B], BF, kind="ExternalInput")
    c_init = nc.dram_tensor("c_init", [128, HC, B], F32, kind="ExternalInput")
    alpha = nc.dram_tensor("alpha", [128, 1], F32, kind="ExternalInput")
    beta = nc.dram_tensor("beta", [128, 1], F32, kind="ExternalInput")
    emb_stream = nc.dram_tensor("emb_stream", [iters, E, COLS], BF, kind="ExternalInput")
    enc_lhsT = nc.dram_tensor("enc_lhsT", [B, S, H], BF, kind="ExternalInput")
    encT_rhs = nc.dram_tensor("encT_rhs", [B, H, S], BF, kind="ExternalInput")
    w_attT = nc.dram_tensor("w_attT", [E, H], BF, kind="ExternalInput")
    b_att_in = nc.dram_tensor("b_att_in", [128, HC], F32, kind="ExternalInput")
    mask_row = nc.dram_tensor("mask_row", [B, 1, S], BF, kind="ExternalInput")
    valid_in = nc.dram_tensor("valid_in", [B, CH, nch], F32, kind="ExternalInput")
    w_fcT = nc.dram_tensor("w_fcT", [2 * H, OUT], BF, kind="ExternalInput")
    b_fc_row = nc.dram_tensor("b_fc_row", [1, OUT], BF, kind="ExternalInput")
    out_d = nc.dram_tensor("out", [B, t_total, OUT], F32, kind="ExternalOutput")

    # ---- DRAM internals ----
    ag_out = nc.dram_tensor("ag_out", [N_CORES, H, COLS], BF, addr_space="Shared")
    dec_dram = nc.dram_tensor("dec_dram", [B, nch, 128, HC * CH], BF)

    groups = [list(range(N_CORES))]

    with tile.TileContext(nc) as tc:
        with (
            tc.tile_pool(name="wpool", bufs=1) as wpool,
            tc.tile_pool(name="spool", bufs=2) as spool,
            tc.tile_pool(name="steppool", bufs=3) as steppool,
            tc.tile_pool(name="pg", bufs=2, space="PSUM") as pg,
            tc.tile_pool(name="pih", bufs=2, space="PSUM") as pih,
            tc.tile_pool(name="ppost", bufs=2, space="PSUM") as ppost,
            tc.tile_pool(name="dpool", bufs=2, space="DRAM") as dpool,
        ):
            # ---- persistent SBUF ----
            wemb_sb = wpool.tile([128, HC, 4 * H], BF, tag="wemb")
            nc.sync.dma_start(wemb_sb[:], w_emb.rearrange("(k p) m -> p k m", p=128))
            win_sb = wpool.tile([128, 2 * HC, 4 * H], BF, tag="win")
            nc.sync.dma_start(win_sb[:], w_in.rearrange("(k p) m -> p k m", p=128))
            whh_sb = wpool.tile([128, HC, 4 * H], BF, tag="whh")
            nc.sync.dma_start(whh_sb[:], w_hh.rearrange("(k p) m -> p k m", p=128))
            biasp_sb = wpool.tile([128, MC], F32, tag="biasp")
            nc.sync.dma_start(biasp_sb[:], biasp[:])
            hinit_sb = wpool.tile([128, HC, B], BF, tag="hinit")
            nc.sync.dma_start(hinit_sb[:], h_init[:])
            cinit_sb = wpool.tile([128, HC, B], F32, tag="cinit")
            nc.sync.dma_start(cinit_sb[:], c_init[:])
            alpha_sb = wpool.tile([128, 1], F32, tag="alpha")
            nc.sync.dma_start(alpha_sb[:], alpha[:])
            beta_sb = wpool.tile([128, 1], F32, tag="beta")
            nc.sync.dma_start(beta_sb[:], beta[:])

            # zero-fill ag_in once, prologue AllGather -> ag_out defined zeros
            zer = spool.tile([128, HC, COLS], BF, tag="zer", bufs=1)
            nc.vector.memset(zer[:], 0.0)
            ag_in0 = dpool.tile([H, COLS], BF, tag="ag_in")
            nc.sync.dma_start(ag_in0.rearrange("(k p) c -> p k c", p=128), zer[:])
            nc.gpsimd.collective_compute(
                "AllGather", mybir.AluOpType.bypass, replica_groups=groups,
                ins=[ag_in0.opt()], outs=[ag_out[:].opt()],
            )

            accum_prev = None
            c_cur = None
            for k in range(iters):
                # ---------- per-iteration stream tiles ----------
                emb_t = spool.tile([128, HC, COLS], BF, tag="emb_t")
                nc.sync.dma_start(
                    emb_t[:], emb_stream[k].rearrange("(k p) c -> p k c", p=128)
                )
                xh_t = spool.tile([128, 2 * HC, COLS], BF, tag="xh_t")
                nc.sync.dma_start(
                    xh_t[:],
                    ag_out[0:2].rearrange("s (k p) c -> p (s k) c", p=128),
                )

                # ---------- ih precompute for this chunk ----------
                ih = spool.tile([128, MC, COLS], BF, tag="ih", bufs=1)
                for m in range(MC):
                    ps = pih.tile([128, COLS], F32, tag="ihps")
                    for kk in range(HC):
                        nc.tensor.matmul(
                            ps[:], wemb_sb[:, kk, m * 128:(m + 1) * 128],
                            emb_t[:, kk, :], start=(kk == 0), stop=False,
                        )
                    for kk in range(2 * HC):
                        nc.tensor.matmul(
                            ps[:], win_sb[:, kk, m * 128:(m + 1) * 128],
                            xh_t[:, kk, :], start=False, stop=(kk == 2 * HC - 1),
                        )
                    nc.vector.tensor_scalar_add(ih[:, m, :], ps[:], biasp_sb[:, m:m + 1])

                # ---------- state carry / blend ----------
                accum = spool.tile([128, HC, (CH + 1) * B], BF, tag="accum")
                if k == 0:
                    nc.vector.tensor_copy(accum[:, :, 0:B], hinit_sb[:])
                    c_new0 = steppool.tile([128, HC, B], F32, tag="c")
                    nc.vector.tensor_copy(c_new0[:], cinit_sb[:])
                    c_cur = c_new0
                elif k == 1:
                    # L1 cores reset to init (alpha=0,beta=1); L0 keep (1,0)
                    t1 = steppool.tile([128, HC, B], F32, tag="blend")
                    nc.vector.tensor_scalar_mul(t1[:], accum_prev[:, :, CH * B:], alpha_sb[:, 0:1])
                    t2 = steppool.tile([128, HC, B], F32, tag="blend")
                    nc.vector.tensor_scalar_mul(t2[:], hinit_sb[:], beta_sb[:, 0:1])
                    nc.vector.tensor_add(accum[:, :, 0:B], t1[:], t2[:])
                    t3 = steppool.tile([128, HC, B], F32, tag="blend")
                    nc.vector.tensor_scalar_mul(t3[:], c_cur[:], alpha_sb[:, 0:1])
                    t4 = steppool.tile([128, HC, B], F32, tag="blend")
                    nc.vector.tensor_scalar_mul(t4[:], cinit_sb[:], beta_sb[:, 0:1])
                    c_new1 = steppool.tile([128, HC, B], F32, tag="c")
                    nc.vector.tensor_add(c_new1[:], t3[:], t4[:])
                    c_cur = c_new1
                else:
                    nc.vector.tensor_copy(accum[:, :, 0:B], accum_prev[:, :, CH * B:])

                # ---------- CH recurrence steps ----------
                for s in range(CH):
                    g_ps = pg.tile([128, MC, B], F32, tag="g")
                    for m in range(MC):
                        for kk in range(HC):
                            nc.tensor.matmul(
                                g_ps[:, m, :],
                                whh_sb[:, kk, m * 128:(m + 1) * 128],
                                accum[:, kk, s * B:(s + 1) * B],
                                start=(kk == 0), stop=(kk == HC - 1),
                            )
                    g_sb = steppool.tile([128, MC, B], F32, tag="g_sb")
                    nc.vector.tensor_add(g_sb[:], g_ps[:], ih[:, :, s * B:(s + 1) * B])
                    sig = steppool.tile([128, 12, B], F32, tag="sig")
                    nc.scalar.activation(sig[:], g_sb[:, 0:12, :], mybir.ActivationFunctionType.Sigmoid)
                    tg = steppool.tile([128, HC, B], F32, tag="tg")
                    nc.scalar.activation(tg[:], g_sb[:, 12:16, :], mybir.ActivationFunctionType.Tanh)
                    m1 = steppool.tile([128, HC, B], F32, tag="m1")
                    nc.vector.tensor_mul(m1[:], sig[:, 4:8, :], c_cur[:])
                    m2 = steppool.tile([128, HC, B], F32, tag="m2")
                    nc.vector.tensor_mul(m2[:], sig[:, 0:4, :], tg[:])
                    c_new = steppool.tile([128, HC, B], F32, tag="c")
                    nc.vector.tensor_add(c_new[:], m1[:], m2[:])
                    tc_t = steppool.tile([128, HC, B], F32, tag="tc")
                    nc.scalar.activation(tc_t[:], c_new[:], mybir.ActivationFunctionType.Tanh)
                    nc.vector.tensor_mul(accum[:, :, (s + 1) * B:(s + 2) * B], sig[:, 8:12, :], tc_t[:])
                    c_cur = c_new

                # ---------- exchange ----------
                ag_in = dpool.tile([H, COLS], BF, tag="ag_in")
                nc.sync.dma_start(
                    ag_in.rearrange("(k p) c -> p k c", p=128), accum[:, :, B:]
                )
                nc.gpsimd.collective_compute(
                    "AllGather", mybir.AluOpType.bypass, replica_groups=groups,
                    ins=[ag_in.opt()], outs=[ag_out[:].opt()],
                )

                # ---------- dec extraction (chunk k-1) ----------
                if k >= 1:
                    s23 = spool.tile([128, 2, HC, COLS], BF, tag="s23", bufs=1)
                    nc.sync.dma_start(
                        s23[:], ag_out[2:4].rearrange("s (k p) c -> p s k c", p=128)
                    )
                    dsum = spool.tile([128, HC, CH, B], BF, tag="dsum", bufs=1)
                    nc.vector.tensor_add(
                        dsum[:],
                        s23[:, 0].rearrange("p k (s b) -> p k s b", b=B),
                        s23[:, 1].rearrange("p k (s b) -> p k s b", b=B),
                    )
                    for lb in range(B):
                        dslice = spool.tile([128, HC, CH], BF, tag="dslice", bufs=2)
                        nc.vector.tensor_copy(dslice[:], dsum[:, :, :, lb])
                        nc.sync.dma_start(
                            dec_dram[lb, k - 1],
                            dslice.rearrange("p a b -> p (a b)"),
                        )
                accum_prev = accum

            # ================= post phase =================
            identity = wpool.tile([128, 128], BF, tag="ident")
            from concourse.masks import make_identity
            make_identity(nc, identity[:])
            ones1 = wpool.tile([1, 128], BF, tag="ones1")
            nc.vector.memset(ones1[:], 1.0)

            wattT_sb = wpool.tile([128, HC, H], BF, tag="wattT")
            nc.sync.dma_start(wattT_sb[:], w_attT.rearrange("(k p) m -> p k m", p=128))
            batt_sb = wpool.tile([128, HC], F32, tag="batt")
            nc.sync.dma_start(batt_sb[:], b_att_in[:])
            wfc_sb = wpool.tile([128, 2 * HC, OUT], BF, tag="wfc")
            nc.sync.dma_start(wfc_sb[:], w_fcT.rearrange("(k p) m -> p k m", p=128))
            bfc_sb = wpool.tile([1, OUT], BF, tag="bfc")
            nc.sync.dma_start(bfc_sb[:], b_fc_row[:])

            for lb in range(B):
                encT_sb = spool.tile([128, HC, S], BF, tag="encT", bufs=1)
                nc.sync.dma_start(
                    encT_sb[:], encT_rhs[lb].rearrange("(k p) s -> p k s", p=128)
                )
                enc_sb = spool.tile([128, SC, H], BF, tag="enc", bufs=1)
                nc.sync.dma_start(
                    enc_sb[:], enc_lhsT[lb].rearrange("(k p) h -> p k h", p=128)
                )
                mask_sb = spool.tile([1, S], BF, tag="mask", bufs=1)
                nc.sync.dma_start(mask_sb[:], mask_row[lb])
                valid_sb = spool.tile([CH, nch], F32, tag="valid", bufs=1)
                nc.sync.dma_start(valid_sb[:], valid_in[lb])
                dec_sb = spool.tile([128, nch, HC, CH], BF, tag="dec_sb", bufs=1)
                nc.sync.dma_start(
                    dec_sb.rearrange("p n a b -> p n (a b)"),
                    dec_dram[lb].rearrange("n p x -> p n x"),
                )

                # enc_projT [H, S]
                epT = spool.tile([128, HC, S], BF, tag="epT", bufs=1)
                for m in range(HC):
                    pp = ppost.tile([128, S], F32, tag="pp")
                    for e in range(HC):
                        nc.tensor.matmul(
                            pp[:], wattT_sb[:, e, m * 128:(m + 1) * 128],
                            encT_sb[:, e, :], start=(e == 0), stop=(e == HC - 1),
                        )
                    nc.vector.tensor_scalar_add(epT[:, m, :], pp[:], batt_sb[:, m:m + 1])

                attT = spool.tile([128, SC, t_total], BF, tag="attT", bufs=1)
                for t in range(nch):
                    sp = ppost.tile([CH, S], F32, tag="pp")
                    for hk in range(HC):
                        nc.tensor.matmul(
                            sp[:], dec_sb[:, t, hk, :],
                            epT[:, hk, :], start=(hk == 0), stop=False,
                        )
                    nc.tensor.matmul(
                        sp[:], ones1[:, 0:CH], mask_sb[:], start=False, stop=True,
                    )
                    mx = steppool.tile([CH, 1], F32, tag="mx")
                    nc.vector.reduce_max(mx[:], sp[:], axis=mybir.AxisListType.X)
                    negmax = steppool.tile([CH, 1], F32, tag="negmax")
                    nc.scalar.mul(negmax[:], mx[:], -1.0)
                    att = steppool.tile([CH, S], BF, tag="att")
                    sumexp = steppool.tile([CH, 1], F32, tag="sumexp")
                    nc.scalar.activation(
                        att[:], sp[:], mybir.ActivationFunctionType.Exp,
                        bias=negmax[:], accum_out=sumexp[:],
                    )
                    recip = steppool.tile([CH, 1], F32, tag="recip")
                    nc.vector.reciprocal(recip[:], sumexp[:])
                    attn = steppool.tile([CH, S], BF, tag="attn")
                    nc.vector.tensor_scalar_mul(attn[:], att[:], recip[:])
                    for sk in range(SC):
                        tp = ppost.tile([128, CH], BF, tag="ptr")
                        nc.tensor.transpose(tp[:], attn[:, sk * 128:(sk + 1) * 128], identity[0:CH, 0:CH])
                        nc.vector.tensor_copy(attT[:, sk, t * CH:(t + 1) * CH], tp[:])

                ctxT = spool.tile([128, HC, t_total], BF, tag="ctxT", bufs=1)
                for m in range(HC):
                    cp = ppost.tile([128, t_total], F32, tag="pp")
                    for sk in range(SC):
                        nc.tensor.matmul(
                            cp[:], enc_sb[:, sk, m * 128:(m + 1) * 128],
                            attT[:, sk, :], start=(sk == 0), stop=(sk == SC - 1),
                        )
                    nc.vector.tensor_copy(ctxT[:, m, :], cp[:])

                for t in range(nch):
                    for nhalf in range(2):
                        ncols = OUT // 2
                        fp = ppost.tile([CH, ncols], F32, tag="pp")
                        for kk in range(2 * HC):
                            lhs = (dec_sb[:, t, kk, :] if kk < HC
                                   else ctxT[:, kk - HC, t * CH:(t + 1) * CH])
                            nc.tensor.matmul(
                                fp[:], lhs, wfc_sb[:, kk, nhalf * ncols:(nhalf + 1) * ncols],
                                start=(kk == 0), stop=False,
                            )
                        nc.tensor.matmul(
                            fp[:], ones1[:, 0:CH], bfc_sb[:, nhalf * ncols:(nhalf + 1) * ncols],
                            start=False, stop=True,
                        )
                        osb = steppool.tile([CH, ncols], F32, tag="osb")
                        nc.vector.tensor_scalar_mul(osb[:], fp[:], valid_sb[:, t:t + 1])
                        nc.sync.dma_start(
                            out_d[lb, t * CH:(t + 1) * CH, nhalf * ncols:(nhalf + 1) * ncols],
                            osb[:],
                        )

    nc.compile()
    return nc


# ---------------- host-side preparation ----------------

def _prep_inputs(inputs, nch=NCH):
    iters = nch + 1
    t_total = nch * CH
    nch_ = nch
    tc_n = t_total // 128
    perm = _gate_perm()

    trg = np.asarray(inputs["trg_inputs"]).astype(np.int64)
    trg_len = np.asarray(inputs["trg_len"]).astype(np.int64)
    enc = _f32(inputs["encoder_outputs"])
    h0 = _f32(inputs["h0"]).reshape(L, 2, B, H)
    c0 = _f32(inputs["c0"]).reshape(L, 2, B, H)
    embed = _f32(inputs["embed"])
    W_ih0 = _f32(inputs["W_ih0"])          # [2, 4H, E]
    W_ih1 = _f32(inputs["W_ih1"])[0]       # [2, 4H, 2H]
    W_hh = _f32(inputs["W_hh"])            # [L, 2, 4H, H]
    b_ih = _f32(inputs["b_ih"])            # [L, 2, 4H]
    b_hh = _f32(inputs["b_hh"])
    W_att = _f32(inputs["W_att"])          # [H, H]
    b_att = _f32(inputs["b_att"])          # [H]
    W_fc = _f32(inputs["W_fc"])            # [OUT, 2H]
    b_fc = _f32(inputs["b_fc"])            # [OUT]

    # embedding stream  [iters, E, COLS]; emb_stream[k,e,s*B+b] = X[b,32k+s,e]
    X = embed[trg[:, :t_total]]                       # [B, t, E]
    es = np.zeros((iters, E, COLS), np.float32)
    xt = X.transpose(2, 1, 0)                         # [E, t, B]
    es[:nch] = (
        xt.reshape(E, nch, CH, B).transpose(1, 0, 2, 3).reshape(nch, E, COLS)
    )
    es = _bf(es)

    # per-cell weights (permuted gate rows)
    cells = [(0, 0), (0, 1), (1, 0), (1, 1)]          # (layer, dir)
    zeros_emb = _bf(np.zeros((E, 4 * H)))
    zeros_in = _bf(np.zeros((2 * H, 4 * H)))

    valid_f = (np.arange(t_total)[None, :] < trg_len[:, None]).astype(np.float32)  # [B,t]
    mask_f = np.where(np.arange(S)[None, :] < trg_len[:, None], 0.0, -1e30).astype(np.float32)

    encT = enc.transpose(0, 2, 1)                     # [B, H, S]

    in_maps = []
    for c in range(N_CORES):
        cell = c % 4
        layer, d = cells[cell]
        if layer == 0:
            wemb = _bf(W_ih0[d][perm].T)              # [E, 4H]
            win = zeros_in
        else:
            wemb = zeros_emb
            win = _bf(W_ih1[d][perm].T)               # [2H, 4H]
        whh = _bf(W_hh[layer, d][perm].T)             # [H, 4H]
        bp = (b_ih[layer, d] + b_hh[layer, d])[perm]  # [4H]
        biasp = _f32(bp.reshape(MC, 128).T)           # [128, MC]
        hin = h0[layer, d]                            # [B, H]
        cin = c0[layer, d]
        h_init = _bf(hin.T.reshape(HC, 128, B).transpose(1, 0, 2))   # [128,HC,B]
        c_init = _f32(cin.T.reshape(HC, 128, B).transpose(1, 0, 2))
        a = 1.0 if layer == 0 else 0.0
        alpha = _f32(np.full((128, 1), a))
        beta = _f32(np.full((128, 1), 1.0 - a))

        bidx = list(range(B))

        m = dict(
            w_emb=wemb, w_in=win, w_hh=whh, biasp=biasp,
            h_init=h_init, c_init=c_init, alpha=alpha, beta=beta,
            emb_stream=es,
            enc_lhsT=_bf(enc[bidx]),                  # [2, S, H]
            encT_rhs=_bf(encT[bidx]),                 # [2, H, S]
            w_attT=_bf(W_att.T),
            b_att_in=_f32(b_att.reshape(HC, 128).T),
            mask_row=_bf(mask_f[bidx][:, None, :]),   # [B,1,S]
            valid_in=_f32(
                valid_f[bidx].reshape(B, nch_, CH).transpose(0, 2, 1)
            ),
            w_fcT=_bf(W_fc.T),                        # [2H, OUT]
            b_fc_row=_bf(b_fc[None, :]),
        )
        in_maps.append(m)
    return in_maps


_NC_CACHE = {}


def kernel(**inputs) -> np.ndarray:
    nch = int(os.environ.get("KERNEL_NCH", NCH))
    if nch not in _NC_CACHE:
        _NC_CACHE[nch] = build_nc(nch)
    nc = _NC_CACHE[nch]
    in_maps = _prep_inputs(inputs, nch)
    r = run_bass_kernel_spmd(nc, in_maps, list(range(N_CORES)))
    return np.asarray(r.results[0]["out"], np.float32)



# revision 12
# speedup vs baseline: 1.6245x; 1.6245x over previous
"""Bass/Trainium2 kernel for nn_Decoder (2-layer bidir-style LSTM decoder
with general attention + fc), distributed over 8 NeuronCores.

v2 architecture (SPMD, one uniform program; per-core behavior differs only
in input DATA):
  - 4 LSTM cells (L0f, L0b, L1f, L1b) -> cores 0..3; cores 4..7 mirror 0..3.
    AllGather runs in two replica groups {0..3},{4..7} carrying 4 slots.
  - Layer 1 lags layer 0 by TWO chunks: the input-to-hidden precompute for
    iteration k+1 (ih = W_emb@emb + W_in@xh + bias) consumes the AllGather
    issued at the end of iteration k-1, so neither the AG nor the ih matmuls
    sit on the critical path. The ih matmuls are emitted interleaved between
    recurrence steps: they execute on the PE during the per-step
    nonlinearity chain (which otherwise leaves the PE idle and HAM-cold).
    ih is split into half-chunk tiles (steps 0-15 / 16-31) so double
    buffering works with the interleaved fill schedule.
  - Per step: gates = ih[:, s] + Whh @ h_{s-1} on PE (weights stationary,
    gates on partitions, batch=16 moving), then the LSTM nonlinearity
    split across ACT (sigmoid/tanh), DVE and Pool engines.
  - dec_t = h1f + h1b extracted from AG slots 2,3 into dec2 DRAM laid out
    [(b, partition) rows x time], one DMA per chunk.
  - Post phase: each core handles only ITS 2 batch elements (batch-parallel
    over 8 cores). The per-core dec rows are fetched with an indirect DMA
    gather using a per-core index tensor (data-driven, keeps the program
    uniform). Attention + fc are done in 128-timestep blocks; fc bias is
    folded in as a K=1 matmul. Host reassembles [16, T, OUT] from the 8
    per-core [2, T, OUT] outputs.

Numerics: bf16 weights/activations with fp32 PSUM accumulation and fp32
cell state c. Optionally (KERNEL_FP8=1) the recurrent weights W_hh are fp8
e4m3 (faster weight loads; slightly higher error).
"""

import os
import sys

sys.path.insert(0, "/opt/trn_rl_repo")

import numpy as np
import ml_dtypes

import concourse.bass as bass
import concourse.mybir as mybir
import concourse.tile as tile
from concourse import bacc
from concourse.bass_utils import run_bass_kernel_spmd

# ---- problem constants (hardcoded per contract) ----
L = 2
H = 512
E = 512
B = 16
T = 512
S = 512
VOCAB = 1001
OUT = 1000

N_CORES = 8
CH = 32                       # timesteps per chunk
NCH = T // CH                 # 16 chunks
LAG = 2                       # L1 runs two chunks behind L0
COLS = CH * B                 # 512 columns per chunk (s-major, b-minor)
HCOLS = COLS // 2             # half-chunk columns (16 steps)
HC = H // 128                 # 4 H-chunks
MC = (4 * H) // 128           # 16 gate M-chunks
SC = S // 128
TC = 4                        # 128-timestep blocks in post phase (T=512)
LBC = 2                       # local batches per core in post phase
BF = mybir.dt.bfloat16
F32 = mybir.dt.float32
I32 = mybir.dt.int32
FP8 = mybir.dt.float8e4

Sig = mybir.ActivationFunctionType.Sigmoid
Tanh = mybir.ActivationFunctionType.Tanh
Exp = mybir.ActivationFunctionType.Exp


# gate permutation: torch order i,f,g,o -> i,f,o,g  (rows of the 4H dim)
def _gate_perm():
    idx = np.arange(4 * H)
    return np.concatenate([idx[0:H], idx[H:2 * H], idx[3 * H:4 * H], idx[2 * H:3 * H]])


def _bf(x):
    return np.ascontiguousarray(np.asarray(x, dtype=np.float32)).astype(ml_dtypes.bfloat16)


def _f32(x):
    return np.ascontiguousarray(np.asarray(x, dtype=np.float32))


def _fp8(x):
    return np.ascontiguousarray(np.asarray(x, dtype=np.float32)).astype(ml_dtypes.float8_e4m3fn)


def build_nc(nch=NCH, fp8=False):
    iters = nch + LAG
    t_total = nch * CH
    whh_dt = FP8 if fp8 else BF
    nc = bacc.Bacc("TRN2", target_bir_lowering=False, debug=False, num_devices=N_CORES)

    # ---- DRAM inputs ----
    w_emb = nc.dram_tensor("w_emb", [E, 4 * H], BF, kind="ExternalInput")
    w_in = nc.dram_tensor("w_in", [2 * H, 4 * H], BF, kind="ExternalInput")
    w_hh = nc.dram_tensor("w_hh", [H, 4 * H], whh_dt, kind="ExternalInput")
    biasp = nc.dram_tensor("biasp", [128, MC], F32, kind="ExternalInput")
    h_init = nc.dram_tensor("h_init", [128, HC, B], BF, kind="ExternalInput")
    c_init = nc.dram_tensor("c_init", [128, HC, B], F32, kind="ExternalInput")
    h_init_eff = nc.dram_tensor("h_init_eff", [128, HC, B], BF, kind="ExternalInput")
    c_init_eff = nc.dram_tensor("c_init_eff", [128, HC, B], F32, kind="ExternalInput")
    alpha = nc.dram_tensor("alpha", [128, 1], F32, kind="ExternalInput")
    emb_stream = nc.dram_tensor("emb_stream", [iters, E, COLS], BF, kind="ExternalInput")
    enc_lhsT = nc.dram_tensor("enc_lhsT", [LBC, S, H], BF, kind="ExternalInput")
    encT_rhs = nc.dram_tensor("encT_rhs", [LBC, H, S], BF, kind="ExternalInput")
    w_attT = nc.dram_tensor("w_attT", [H, H], BF, kind="ExternalInput")
    b_att_in = nc.dram_tensor("b_att_in", [128, HC], F32, kind="ExternalInput")
    mask_row = nc.dram_tensor("mask_row", [LBC, 1, S], BF, kind="ExternalInput")
    valid_in = nc.dram_tensor("valid_in", [128, LBC * TC], F32, kind="ExternalInput")
    w_fcT = nc.dram_tensor("w_fcT", [2 * H, OUT], BF, kind="ExternalInput")
    b_fc_row = nc.dram_tensor("b_fc_row", [1, OUT], BF, kind="ExternalInput")
    dec_idx = nc.dram_tensor("dec_idx", [128, LBC], I32, kind="ExternalInput")
    out_d = nc.dram_tensor("out", [LBC, t_total, OUT], F32, kind="ExternalOutput")

    # ---- DRAM internals ----
    ag_out = nc.dram_tensor("ag_out", [N_CORES, H, COLS], BF, addr_space="Shared")
    dec2 = nc.dram_tensor("dec2", [B, 128, nch * HC * CH], BF)

    groups = [list(range(N_CORES))]

    with tile.TileContext(nc) as tc:
        with (
            tc.tile_pool(name="wpool", bufs=1) as wpool,
            tc.tile_pool(name="dpool", bufs=2, space="DRAM") as dpool,
        ):
            # ---- persistent SBUF (scan weights/state) ----
            wemb_sb = wpool.tile([128, HC, 4 * H], BF, tag="wemb")
            nc.sync.dma_start(wemb_sb[:], w_emb.rearrange("(k p) m -> p k m", p=128))
            win_sb = wpool.tile([128, 2 * HC, 4 * H], BF, tag="win")
            nc.sync.dma_start(win_sb[:], w_in.rearrange("(k p) m -> p k m", p=128))
            whh_sb = wpool.tile([128, HC, 4 * H], whh_dt, tag="whh")
            nc.sync.dma_start(whh_sb[:], w_hh.rearrange("(k p) m -> p k m", p=128))
            biasp_sb = wpool.tile([128, MC], F32, tag="biasp")
            nc.sync.dma_start(biasp_sb[:], biasp[:])
            hinit_sb = wpool.tile([128, HC, B], BF, tag="hinit")
            nc.sync.dma_start(hinit_sb[:], h_init[:])
            cinit_sb = wpool.tile([128, HC, B], F32, tag="cinit")
            nc.sync.dma_start(cinit_sb[:], c_init[:])
            hinite_sb = wpool.tile([128, HC, B], BF, tag="hinite")
            nc.sync.dma_start(hinite_sb[:], h_init_eff[:])
            cinite_sb = wpool.tile([128, HC, B], F32, tag="cinite")
            nc.sync.dma_start(cinite_sb[:], c_init_eff[:])
            alpha_sb = wpool.tile([128, 1], F32, tag="alpha")
            nc.sync.dma_start(alpha_sb[:], alpha[:])

            # =================== scan phase ===================
            with (
                tc.tile_pool(name="spool", bufs=2) as spool,
                tc.tile_pool(name="steppool", bufs=3) as steppool,
                tc.tile_pool(name="pg", bufs=2, space="PSUM") as pg,
                tc.tile_pool(name="pih", bufs=2, space="PSUM") as pih,
            ):
                # zero-fill ag_in once; prologue AllGather -> defined zeros
                zer = spool.tile([128, HC, COLS], BF, tag="zer", bufs=1)
                nc.vector.memset(zer[:], 0.0)
                ag_in0 = dpool.tile([H, COLS], BF, tag="ag_in")
                nc.sync.dma_start(ag_in0.rearrange("(k p) c -> p k c", p=128), zer[:])
                nc.gpsimd.collective_compute(
                    "AllGather", mybir.AluOpType.bypass, replica_groups=groups,
                    ins=[ag_in0.opt()], outs=[ag_out[:].opt()],
                )

                def new_xh():
                    t = spool.tile([128, 2 * HC, COLS], BF, tag="xh", bufs=3, name="xh")
                    nc.sync.dma_start(
                        t[:], ag_out[0:2].rearrange("s (k p) c -> p (s k) c", p=128)
                    )
                    return t

                def new_emb(k):
                    t = spool.tile([128, HC, COLS], BF, tag="emb", bufs=3, name="emb")
                    nc.sync.dma_start(
                        t[:], emb_stream[k].rearrange("(k p) c -> p k c", p=128)
                    )
                    return t

                def ih_group(ihdst, m, emb_t, xh_t, half):
                    # ih for gate chunk m over one half-chunk of columns
                    c0, c1 = half * HCOLS, (half + 1) * HCOLS
                    ps = pih.tile([128, HCOLS], F32, tag="ihps", name="ihps")
                    for kk in range(HC):
                        nc.tensor.matmul(
                            ps[:], wemb_sb[:, kk, m * 128:(m + 1) * 128],
                            emb_t[:, kk, c0:c1], start=(kk == 0), stop=False,
                        )
                    for kk in range(2 * HC):
                        nc.tensor.matmul(
                            ps[:], win_sb[:, kk, m * 128:(m + 1) * 128],
                            xh_t[:, kk, c0:c1], start=False, stop=(kk == 2 * HC - 1),
                        )
                    nc.scalar.add(ihdst[:, m, :], ps[:], biasp_sb[:, m:m + 1])

                def new_ih(tag):
                    return spool.tile([128, MC, HCOLS], BF, tag=tag, bufs=2, name=tag)

                # prologue: emb/xh and full ih for iteration 0
                emb_cur = new_emb(0)
                xh_cur = new_xh()
                ihA_cur = new_ih("ihA")
                ihB_cur = new_ih("ihB")
                for m in range(MC):
                    ih_group(ihA_cur, m, emb_cur, xh_cur, 0)
                for m in range(MC):
                    ih_group(ihB_cur, m, emb_cur, xh_cur, 1)

                accum_prev = None
                c_cur = None
                for k in range(iters):
                    last = k == iters - 1
                    if not last:
                        emb_next = new_emb(k + 1)
                        xh_next = new_xh()
                        ihA_next = new_ih("ihA")
                        ihB_next = new_ih("ihB")

                    # dec extraction for chunk k-3 (reads AG(k-1) output;
                    # emitted before this iteration's AG write -> safe WAR)
                    if k >= LAG + 1:
                        s23 = spool.tile([128, 2, HC, COLS], BF, tag="s23", bufs=2)
                        nc.sync.dma_start(
                            s23[:], ag_out[2:4].rearrange("s (k p) c -> p s k c", p=128)
                        )
                        dsum = spool.tile([128, B, HC, CH], BF, tag="dsum", bufs=2)
                        nc.gpsimd.tensor_add(
                            dsum[:],
                            s23[:, 0].rearrange("p k (s b) -> p b k s", b=B),
                            s23[:, 1].rearrange("p k (s b) -> p b k s", b=B),
                        )
                        ck = k - LAG - 1
                        nc.sync.dma_start(
                            dec2[:, :, ck * HC * CH:(ck + 1) * HC * CH].rearrange(
                                "b p x -> p b x"
                            ),
                            dsum.rearrange("p b k s -> p b (k s)"),
                        )

                    # ---------- state carry / blend ----------
                    accum = spool.tile([128, HC, (CH + 1) * B], BF, tag="accum")
                    if k == 0:
                        nc.vector.tensor_copy(accum[:, :, 0:B], hinit_sb[:])
                        c0t = steppool.tile([128, HC, B], F32, tag="c")
                        nc.vector.tensor_copy(c0t[:], cinit_sb[:])
                        c_cur = c0t
                    elif k == LAG:
                        # L1 cores reset to init (alpha=0); L0 keep (alpha=1)
                        nc.vector.scalar_tensor_tensor(
                            accum[:, :, 0:B], accum_prev[:, :, CH * B:],
                            alpha_sb[:, 0:1], hinite_sb[:],
                            mybir.AluOpType.mult, mybir.AluOpType.add,
                        )
                        cbl = steppool.tile([128, HC, B], F32, tag="c")
                        nc.vector.scalar_tensor_tensor(
                            cbl[:], c_cur[:], alpha_sb[:, 0:1], cinite_sb[:],
                            mybir.AluOpType.mult, mybir.AluOpType.add,
                        )
                        c_cur = cbl
                    else:
                        nc.vector.tensor_copy(accum[:, :, 0:B], accum_prev[:, :, CH * B:])

                    # ---------- CH recurrence steps with interleaved ih fill ----------
                    for s in range(CH):
                        ihs = ihA_cur if s < CH // 2 else ihB_cur
                        scol = (s % (CH // 2)) * B
                        g_ps = pg.tile([128, MC, B], F32, tag="g")
                        # gate-group order g,i,f,o: each group's nonlinearity
                        # starts as soon as its 16 MMs land, pipelining most
                        # of the chain under the MM burst.
                        for m in (12, 13, 14, 15, 0, 1, 2, 3, 4, 5, 6, 7, 8, 9, 10, 11):
                            for kk in range(HC):
                                nc.tensor.matmul(
                                    g_ps[:, m, :],
                                    whh_sb[:, kk, m * 128:(m + 1) * 128],
                                    accum[:, kk, s * B:(s + 1) * B],
                                    start=(kk == 0), stop=(kk == HC - 1),
                                )
                        gadd = steppool.tile([128, MC, B], F32, tag="g_sb")
                        sig = steppool.tile([128, 12, B], F32, tag="sig")
                        tg = steppool.tile([128, HC, B], F32, tag="tg")
                        # g group
                        nc.vector.tensor_tensor(
                            gadd[:, 12:16, :], g_ps[:, 12:16, :],
                            ihs[:, 12:16, scol:scol + B], mybir.AluOpType.add,
                        )
                        nc.scalar.activation(tg[:], gadd[:, 12:16, :], Tanh)
                        # i group
                        nc.vector.tensor_tensor(
                            gadd[:, 0:4, :], g_ps[:, 0:4, :],
                            ihs[:, 0:4, scol:scol + B], mybir.AluOpType.add,
                        )
                        nc.scalar.activation(sig[:, 0:4, :], gadd[:, 0:4, :], Sig)
                        m2 = steppool.tile([128, HC, B], F32, tag="m2")
                        nc.gpsimd.tensor_mul(m2[:], sig[:, 0:4, :], tg[:])
                        # f group
                        nc.vector.tensor_tensor(
                            gadd[:, 4:8, :], g_ps[:, 4:8, :],
                            ihs[:, 4:8, scol:scol + B], mybir.AluOpType.add,
                        )
                        nc.scalar.activation(sig[:, 4:8, :], gadd[:, 4:8, :], Sig)
                        m1 = steppool.tile([128, HC, B], F32, tag="m1")
                        nc.vector.tensor_mul(m1[:], sig[:, 4:8, :], c_cur[:])
                        # o group
                        nc.vector.tensor_tensor(
                            gadd[:, 8:12, :], g_ps[:, 8:12, :],
                            ihs[:, 8:12, scol:scol + B], mybir.AluOpType.add,
                        )
                        nc.scalar.activation(sig[:, 8:12, :], gadd[:, 8:12, :], Sig)
                        # state update
                        c_new = steppool.tile([128, HC, B], F32, tag="c")
                        nc.gpsimd.tensor_add(c_new[:], m1[:], m2[:])
                        tc_t = steppool.tile([128, HC, B], F32, tag="tc")
                        nc.scalar.activation(tc_t[:], c_new[:], Tanh)
                        nc.vector.tensor_mul(
                            accum[:, :, (s + 1) * B:(s + 2) * B], sig[:, 8:12, :], tc_t[:]
                        )
                        c_cur = c_new
                        # filler: ih matmuls for the next iteration, emitted
                        # last so their bias-adds queue behind the chain ops.
                        if not last:
                            if 8 <= s < 16:
                                for m in (2 * (s - 8), 2 * (s - 8) + 1):
                                    ih_group(ihB_next, m, emb_next, xh_next, 1)
                            elif s >= 16:
                                ih_group(ihA_next, s - 16, emb_next, xh_next, 0)

                    # ---------- exchange ----------
                    ag_in = dpool.tile([H, COLS], BF, tag="ag_in")
                    nc.sync.dma_start(
                        ag_in.rearrange("(k p) c -> p k c", p=128), accum[:, :, B:]
                    )
                    nc.gpsimd.collective_compute(
                        "AllGather", mybir.AluOpType.bypass, replica_groups=groups,
                        ins=[ag_in.opt()], outs=[ag_out[:].opt()],
                    )
                    accum_prev = accum
                    if not last:
                        emb_cur, xh_cur = emb_next, xh_next
                        ihA_cur, ihB_cur = ihA_next, ihB_next

                # tail: AG(iters-1) carries L1's final chunk (nch-1)
                s23 = spool.tile([128, 2, HC, COLS], BF, tag="s23", bufs=2)
                nc.sync.dma_start(
                    s23[:], ag_out[2:4].rearrange("s (k p) c -> p s k c", p=128)
                )
                dsum = spool.tile([128, B, HC, CH], BF, tag="dsum", bufs=2)
                nc.gpsimd.tensor_add(
                    dsum[:],
                    s23[:, 0].rearrange("p k (s b) -> p b k s", b=B),
                    s23[:, 1].rearrange("p k (s b) -> p b k s", b=B),
                )
                ck = nch - 1
                nc.sync.dma_start(
                    dec2[:, :, ck * HC * CH:(ck + 1) * HC * CH].rearrange(
                        "b p x -> p b x"
                    ),
                    dsum.rearrange("p b k s -> p b (k s)"),
                )

            # =================== post phase ===================
            with (
                tc.tile_pool(name="ppool", bufs=1) as ppool,
                tc.tile_pool(name="pstep", bufs=3) as pstep,
                tc.tile_pool(name="ppost", bufs=2, space="PSUM") as ppost,
            ):
                from concourse.masks import make_identity
                identity = ppool.tile([128, 128], BF, tag="ident")
                make_identity(nc, identity[:])
                ones1 = ppool.tile([1, 128], BF, tag="ones1")
                nc.vector.memset(ones1[:], 1.0)

                wattT_sb = ppool.tile([128, HC, H], BF, tag="wattT")
                nc.sync.dma_start(wattT_sb[:], w_attT.rearrange("(k p) m -> p k m", p=128))
                batt_sb = ppool.tile([128, HC], F32, tag="batt")
                nc.sync.dma_start(batt_sb[:], b_att_in[:])
                wfc_sb = ppool.tile([128, 2 * HC, OUT], BF, tag="wfc")
                nc.sync.dma_start(wfc_sb[:], w_fcT.rearrange("(k p) m -> p k m", p=128))
                bfc_sb = ppool.tile([1, OUT], BF, tag="bfc")
                nc.sync.dma_start(bfc_sb[:], b_fc_row[:])
                idx_sb = ppool.tile([128, LBC], I32, tag="idx")
                nc.sync.dma_start(idx_sb[:], dec_idx[:])
                valid_sb = ppool.tile([128, LBC * TC], F32, tag="valid")
                nc.sync.dma_start(valid_sb[:], valid_in[:])

                nhalf_cols = OUT // 2
                for j in range(LBC):
                    dec_sb = ppool.tile([128, nch * HC * CH], BF, tag="dec", bufs=2)
                    nc.gpsimd.indirect_dma_start(
                        out=dec_sb[:],
                        out_offset=None,
                        in_=dec2.rearrange("b p x -> (b p) x"),
                        in_offset=bass.IndirectOffsetOnAxis(ap=idx_sb[:, j:j + 1], axis=0),
                    )
                    # reorder to hk-major so matmul lhsT slices have one free dim
                    dec_kb = ppool.tile([128, HC, nch * CH], BF, tag="deck", bufs=2)
                    nc.gpsimd.tensor_copy(
                        dec_kb.rearrange("p k (n s) -> p k n s", s=CH),
                        dec_sb.rearrange("p (n k s) -> p k n s", k=HC, s=CH),
                    )
                    encT_sb = ppool.tile([128, HC, S], BF, tag="encT", bufs=2)
                    nc.sync.dma_start(
                        encT_sb[:], encT_rhs[j].rearrange("(k p) s -> p k s", p=128)
                    )
                    enc_sb = ppool.tile([128, SC, H], BF, tag="enc", bufs=2)
                    nc.sync.dma_start(
                        enc_sb[:], enc_lhsT[j].rearrange("(k p) h -> p k h", p=128)
                    )
                    mask_sb = ppool.tile([1, S], BF, tag="mask", bufs=2)
                    nc.sync.dma_start(mask_sb[:], mask_row[j])

                    # enc_projT [H, S]
                    epT = ppool.tile([128, HC, S], BF, tag="epT", bufs=2)
                    for m in range(HC):
                        pp = ppost.tile([128, S], F32, tag="pp")
                        for e in range(HC):
                            nc.tensor.matmul(
                                pp[:], wattT_sb[:, e, m * 128:(m + 1) * 128],
                                encT_sb[:, e, :], start=(e == 0), stop=(e == HC - 1),
                            )
                        nc.vector.tensor_scalar_add(epT[:, m, :], pp[:], batt_sb[:, m:m + 1])

                    # attention over 128-timestep blocks
                    attT = ppool.tile([128, SC, T], BF, tag="attT", bufs=2)
                    for tb in range(TC):
                        sp = ppost.tile([128, S], F32, tag="pp")
                        for hk in range(HC):
                            nc.tensor.matmul(
                                sp[:], dec_kb[:, hk, tb * 128:(tb + 1) * 128],
                                epT[:, hk, :], start=(hk == 0), stop=False,
                            )
                        nc.tensor.matmul(
                            sp[:], ones1[0:1, :], mask_sb[:], start=False, stop=True,
                        )
                        mx = pstep.tile([128, 1], F32, tag="mx")
                        nc.vector.reduce_max(mx[:], sp[:], axis=mybir.AxisListType.X)
                        negmax = pstep.tile([128, 1], F32, tag="negmax")
                        nc.scalar.mul(negmax[:], mx[:], -1.0)
                        att = pstep.tile([128, S], BF, tag="att")
                        sumexp = pstep.tile([128, 1], F32, tag="sumexp")
                        nc.scalar.activation(
                            att[:], sp[:], Exp, bias=negmax[:], accum_out=sumexp[:],
                        )
                        recip = pstep.tile([128, 1], F32, tag="recip")
                        nc.vector.reciprocal(recip[:], sumexp[:])
                        attn = pstep.tile([128, S], BF, tag="attn")
                        nc.vector.tensor_scalar_mul(attn[:], att[:], recip[:])
                        for sk in range(SC):
                            tp = ppost.tile([128, 128], BF, tag="ptr")
                            nc.tensor.transpose(
                                tp[:], attn[:, sk * 128:(sk + 1) * 128], identity[:]
                            )
                            nc.vector.tensor_copy(
                                attT[:, sk, tb * 128:(tb + 1) * 128], tp[:]
                            )

                    # context ctxT [H, T]
                    ctxT = ppool.tile([128, HC, T], BF, tag="ctxT", bufs=2)
                    for m in range(HC):
                        cp = ppost.tile([128, T], F32, tag="pp")
                        for sk in range(SC):
                            nc.tensor.matmul(
                                cp[:], enc_sb[:, sk, m * 128:(m + 1) * 128],
                                attT[:, sk, :], start=(sk == 0), stop=(sk == SC - 1),
                            )
                        nc.vector.tensor_copy(ctxT[:, m, :], cp[:])

                    # fc: [128 t, OUT] blocks
                    for tb in range(TC):
                        for nh in range(2):
                            fp = ppost.tile([128, nhalf_cols], F32, tag="fp")
                            for kk in range(2 * HC):
                                lhs = (dec_kb[:, kk, tb * 128:(tb + 1) * 128] if kk < HC
                                       else ctxT[:, kk - HC, tb * 128:(tb + 1) * 128])
                                nc.tensor.matmul(
                                    fp[:], lhs,
                                    wfc_sb[:, kk, nh * nhalf_cols:(nh + 1) * nhalf_cols],
                                    start=(kk == 0), stop=False,
                                )
                            nc.tensor.matmul(
                                fp[:], ones1[0:1, :],
                                bfc_sb[:, nh * nhalf_cols:(nh + 1) * nhalf_cols],
                                start=False, stop=True,
                            )
                            osb = pstep.tile([128, nhalf_cols], F32, tag="osb")
                            nc.vector.tensor_scalar_mul(
                                osb[:], fp[:], valid_sb[:, j * TC + tb:j * TC + tb + 1]
                            )
                            nc.sync.dma_start(
                                out_d[j, tb * 128:(tb + 1) * 128,
                                      nh * nhalf_cols:(nh + 1) * nhalf_cols],
                                osb[:],
                            )

    nc.compile()
    return nc


# ---------------- host-side preparation ----------------

def _prep_inputs(inputs, nch=NCH, fp8=False):
    iters = nch + LAG
    t_total = nch * CH
    perm = _gate_perm()

    trg = np.asarray(inputs["trg_inputs"]).astype(np.int64)
    trg_len = np.asarray(inputs["trg_len"]).astype(np.int64)
    enc = _f32(inputs["encoder_outputs"])
    h0 = _f32(inputs["h0"]).reshape(L, 2, B, H)
    c0 = _f32(inputs["c0"]).reshape(L, 2, B, H)
    embed = _f32(inputs["embed"])
    W_ih0 = _f32(inputs["W_ih0"])          # [2, 4H, E]
    W_ih1 = _f32(inputs["W_ih1"])[0]       # [2, 4H, 2H]
    W_hh = _f32(inputs["W_hh"])            # [L, 2, 4H, H]
    b_ih = _f32(inputs["b_ih"])            # [L, 2, 4H]
    b_hh = _f32(inputs["b_hh"])
    W_att = _f32(inputs["W_att"])          # [H, H]
    b_att = _f32(inputs["b_att"])          # [H]
    W_fc = _f32(inputs["W_fc"])            # [OUT, 2H]
    b_fc = _f32(inputs["b_fc"])            # [OUT]

    # embedding stream [iters, E, COLS]; emb_stream[k,e,s*B+b] = X[b,32k+s,e]
    X = embed[trg[:, :t_total]]                       # [B, t, E]
    es = np.zeros((iters, E, COLS), np.float32)
    xt = X.transpose(2, 1, 0)                         # [E, t, B]
    es[:nch] = (
        xt.reshape(E, nch, CH, B).transpose(1, 0, 2, 3).reshape(nch, E, COLS)
    )
    es = _bf(es)

    cells = [(0, 0), (0, 1), (1, 0), (1, 1)]          # (layer, dir)
    zeros_emb = _bf(np.zeros((E, 4 * H)))
    zeros_in = _bf(np.zeros((2 * H, 4 * H)))

    valid_f = (np.arange(t_total)[None, :] < trg_len[:, None]).astype(np.float32)
    mask_f = np.where(np.arange(S)[None, :] < trg_len[:, None], 0.0, -1e30).astype(np.float32)
    encT = enc.transpose(0, 2, 1)                     # [B, H, S]

    pidx = np.arange(128)

    in_maps = []
    for c in range(N_CORES):
        cell = c % 4
        layer, d = cells[cell]
        if layer == 0:
            wemb = _bf(W_ih0[d][perm].T)              # [E, 4H]
            win = zeros_in
            a = 1.0
        else:
            wemb = zeros_emb
            win = _bf(W_ih1[d][perm].T)               # [2H, 4H]
            a = 0.0
        whh_np = W_hh[layer, d][perm].T               # [H, 4H]
        whh = _fp8(whh_np) if fp8 else _bf(whh_np)
        bp = (b_ih[layer, d] + b_hh[layer, d])[perm]
        biasp = _f32(bp.reshape(MC, 128).T)           # [128, MC]
        hin = h0[layer, d]                            # [B, H]
        cin = c0[layer, d]
        h_init = hin.T.reshape(HC, 128, B).transpose(1, 0, 2)   # [128,HC,B]
        c_init = cin.T.reshape(HC, 128, B).transpose(1, 0, 2)

        # post-phase batch pair for this core
        lbs = [2 * c, 2 * c + 1]

        didx = np.stack([lbs[0] * 128 + pidx, lbs[1] * 128 + pidx], axis=1).astype(np.int32)
        vpt = np.zeros((128, LBC * TC), np.float32)
        for j in range(LBC):
            for tb in range(TC):
                vpt[:, j * TC + tb] = valid_f[lbs[j], tb * 128:(tb + 1) * 128]

        m = dict(
            w_emb=wemb, w_in=win, w_hh=whh, biasp=biasp,
            h_init=_bf(h_init), c_init=_f32(c_init),
            h_init_eff=_bf(h_init * (1.0 - a)), c_init_eff=_f32(c_init * (1.0 - a)),
            alpha=_f32(np.full((128, 1), a)),
            emb_stream=es,
            enc_lhsT=_bf(enc[lbs]),                   # [2, S, H]
            encT_rhs=_bf(encT[lbs]),                  # [2, H, S]
            w_attT=_bf(W_att.T),
            b_att_in=_f32(b_att.reshape(HC, 128).T),
            mask_row=_bf(mask_f[lbs][:, None, :]),    # [2,1,S]
            valid_in=vpt,
            w_fcT=_bf(W_fc.T),                        # [2H, OUT]
            b_fc_row=_bf(b_fc[None, :]),
            dec_idx=didx,
        )
        in_maps.append(m)
    return in_maps


_NC_CACHE = {}


def kernel(**inputs) -> np.ndarray:
    nch = int(os.environ.get("KERNEL_NCH", NCH))
    fp8 = os.environ.get("KERNEL_FP8", "0") == "1"
    key = (nch, fp8)
    if key not in _NC_CACHE:
        _NC_CACHE[key] = build_nc(nch, fp8)
    nc = _NC_CACHE[key]
    in_maps = _prep_inputs(inputs, nch, fp8)
    r = run_bass_kernel_spmd(nc, in_maps, list(range(N_CORES)))
    t_total = nch * CH
    full = np.zeros((B, t_total, OUT), np.float32)
    for c in range(N_CORES):
        o = np.asarray(r.results[c]["out"], np.float32)
        full[2 * c] = o[0]
        full[2 * c + 1] = o[1]
    return full


# revision 14
# speedup vs baseline: 1.9739x; 1.2151x over previous
"""Bass/Trainium2 kernel for nn_Decoder (2-layer bidir-style LSTM decoder
with general attention + fc), distributed over 8 NeuronCores.

v2 architecture (SPMD, one uniform program; per-core behavior differs only
in input DATA):
  - 4 LSTM cells (L0f, L0b, L1f, L1b) -> cores 0..3; cores 4..7 mirror 0..3.
    AllGather runs in two replica groups {0..3},{4..7} carrying 4 slots.
  - Layer 1 lags layer 0 by TWO chunks: the input-to-hidden precompute for
    iteration k+1 (ih = W_emb@emb + W_in@xh + bias) consumes the AllGather
    issued at the end of iteration k-1, so neither the AG nor the ih matmuls
    sit on the critical path. The ih matmuls are emitted interleaved between
    recurrence steps: they execute on the PE during the per-step
    nonlinearity chain (which otherwise leaves the PE idle and HAM-cold).
    ih is split into half-chunk tiles (steps 0-15 / 16-31) so double
    buffering works with the interleaved fill schedule.
  - Per step: gates = ih[:, s] + Whh @ h_{s-1} on PE (weights stationary,
    gates on partitions, batch=16 moving), then the LSTM nonlinearity
    split across ACT (sigmoid/tanh), DVE and Pool engines.
  - dec_t = h1f + h1b extracted from AG slots 2,3 into dec2 DRAM laid out
    [(b, partition) rows x time], one DMA per chunk.
  - Post phase: each core handles only ITS 2 batch elements (batch-parallel
    over 8 cores). The per-core dec rows are fetched with an indirect DMA
    gather using a per-core index tensor (data-driven, keeps the program
    uniform). Attention + fc are done in 128-timestep blocks; fc bias is
    folded in as a K=1 matmul. Host reassembles [16, T, OUT] from the 8
    per-core [2, T, OUT] outputs.

Numerics: bf16 weights/activations with fp32 PSUM accumulation and fp32
cell state c. Optionally (KERNEL_FP8=1) the recurrent weights W_hh are fp8
e4m3 (faster weight loads; slightly higher error).
"""

import os
import sys

sys.path.insert(0, "/opt/trn_rl_repo")

import numpy as np
import ml_dtypes

import concourse.bass as bass
import concourse.mybir as mybir
import concourse.tile as tile
from concourse import bacc
from concourse.bass_utils import run_bass_kernel_spmd

# ---- problem constants (hardcoded per contract) ----
L = 2
H = 512
E = 512
B = 16
T = 512
S = 512
VOCAB = 1001
OUT = 1000

N_CORES = 8
CH = 32                       # timesteps per chunk
NCH = T // CH                 # 16 chunks
LAG = 2                       # L1 runs two chunks behind L0
COLS = CH * B                 # 512 columns per chunk (s-major, b-minor)
HCOLS = COLS // 2             # half-chunk columns (16 steps)
HC = H // 128                 # 4 H-chunks
MC = (4 * H) // 128           # 16 gate M-chunks
SC = S // 128
TC = 4                        # 128-timestep blocks in post phase (T=512)
LBC = 2                       # local batches per core in post phase
BF = mybir.dt.bfloat16
F32 = mybir.dt.float32
I32 = mybir.dt.int32
FP8 = mybir.dt.float8e4

Sig = mybir.ActivationFunctionType.Sigmoid
Tanh = mybir.ActivationFunctionType.Tanh
Exp = mybir.ActivationFunctionType.Exp


# gate permutation: torch order i,f,g,o -> i,f,o,g  (rows of the 4H dim)
def _gate_perm():
    idx = np.arange(4 * H)
    return np.concatenate([idx[0:H], idx[H:2 * H], idx[3 * H:4 * H], idx[2 * H:3 * H]])


def _bf(x):
    return np.ascontiguousarray(np.asarray(x, dtype=np.float32)).astype(ml_dtypes.bfloat16)


def _f32(x):
    return np.ascontiguousarray(np.asarray(x, dtype=np.float32))


def _fp8(x):
    return np.ascontiguousarray(np.asarray(x, dtype=np.float32)).astype(ml_dtypes.float8_e4m3fn)


def build_nc(nch=NCH, fp8=False):
    iters = nch + LAG
    t_total = nch * CH
    whh_dt = FP8 if fp8 else BF
    nc = bacc.Bacc("TRN2", target_bir_lowering=False, debug=False, num_devices=N_CORES)

    # ---- DRAM inputs ----
    w_emb = nc.dram_tensor("w_emb", [E, 4 * H], BF, kind="ExternalInput")
    w_in = nc.dram_tensor("w_in", [2 * H, 4 * H], BF, kind="ExternalInput")
    w_hh = nc.dram_tensor("w_hh", [H, 4 * H], whh_dt, kind="ExternalInput")
    biasp = nc.dram_tensor("biasp", [128, MC], F32, kind="ExternalInput")
    h_init = nc.dram_tensor("h_init", [128, HC, B], BF, kind="ExternalInput")
    c_init = nc.dram_tensor("c_init", [128, HC, B], F32, kind="ExternalInput")
    h_init_eff = nc.dram_tensor("h_init_eff", [128, HC, B], BF, kind="ExternalInput")
    c_init_eff = nc.dram_tensor("c_init_eff", [128, HC, B], F32, kind="ExternalInput")
    alpha = nc.dram_tensor("alpha", [128, 1], F32, kind="ExternalInput")
    emb_stream = nc.dram_tensor("emb_stream", [iters, E, COLS], BF, kind="ExternalInput")
    enc_lhsT = nc.dram_tensor("enc_lhsT", [LBC, S, H], BF, kind="ExternalInput")
    encT_rhs = nc.dram_tensor("encT_rhs", [LBC, H, S], BF, kind="ExternalInput")
    w_attT = nc.dram_tensor("w_attT", [H, H], BF, kind="ExternalInput")
    b_att_in = nc.dram_tensor("b_att_in", [128, HC], F32, kind="ExternalInput")
    mask_row = nc.dram_tensor("mask_row", [LBC, 1, S], BF, kind="ExternalInput")
    valid_in = nc.dram_tensor("valid_in", [128, LBC * TC], F32, kind="ExternalInput")
    w_fcT = nc.dram_tensor("w_fcT", [2 * H, OUT], BF, kind="ExternalInput")
    b_fc_row = nc.dram_tensor("b_fc_row", [1, OUT], BF, kind="ExternalInput")
    dec_idx = nc.dram_tensor("dec_idx", [128, LBC], I32, kind="ExternalInput")
    out_d = nc.dram_tensor("out", [LBC, t_total, OUT], F32, kind="ExternalOutput")

    # ---- DRAM internals ----
    ag_out = nc.dram_tensor("ag_out", [N_CORES, H, COLS], BF, addr_space="Shared")
    dec2 = nc.dram_tensor("dec2", [B, 128, nch * HC * CH], BF)

    groups = [list(range(N_CORES))]

    with tile.TileContext(nc) as tc:
        with (
            tc.tile_pool(name="wpool", bufs=1) as wpool,
            tc.tile_pool(name="dpool", bufs=2, space="DRAM") as dpool,
        ):
            # ---- persistent SBUF (scan weights/state) ----
            wemb_sb = wpool.tile([128, HC, 4 * H], BF, tag="wemb")
            nc.sync.dma_start(wemb_sb[:], w_emb.rearrange("(k p) m -> p k m", p=128))
            win_sb = wpool.tile([128, 2 * HC, 4 * H], BF, tag="win")
            nc.sync.dma_start(win_sb[:], w_in.rearrange("(k p) m -> p k m", p=128))
            whh_sb = wpool.tile([128, HC, 4 * H], whh_dt, tag="whh")
            nc.sync.dma_start(whh_sb[:], w_hh.rearrange("(k p) m -> p k m", p=128))
            biasp_sb = wpool.tile([128, MC], F32, tag="biasp")
            nc.sync.dma_start(biasp_sb[:], biasp[:])
            hinit_sb = wpool.tile([128, HC, B], BF, tag="hinit")
            nc.sync.dma_start(hinit_sb[:], h_init[:])
            cinit_sb = wpool.tile([128, HC, B], F32, tag="cinit")
            nc.sync.dma_start(cinit_sb[:], c_init[:])
            hinite_sb = wpool.tile([128, HC, B], BF, tag="hinite")
            nc.sync.dma_start(hinite_sb[:], h_init_eff[:])
            cinite_sb = wpool.tile([128, HC, B], F32, tag="cinite")
            nc.sync.dma_start(cinite_sb[:], c_init_eff[:])
            alpha_sb = wpool.tile([128, 1], F32, tag="alpha")
            nc.sync.dma_start(alpha_sb[:], alpha[:])

            # =================== scan phase ===================
            with (
                tc.tile_pool(name="spool", bufs=2) as spool,
                tc.tile_pool(name="steppool", bufs=3) as steppool,
                tc.tile_pool(name="pg", bufs=2, space="PSUM") as pg,
                tc.tile_pool(name="pih", bufs=2, space="PSUM") as pih,
            ):
                # zero-fill ag_in once; prologue AllGather -> defined zeros
                zer = spool.tile([128, HC, COLS], BF, tag="zer", bufs=1)
                nc.vector.memset(zer[:], 0.0)
                ag_in0 = dpool.tile([H, COLS], BF, tag="ag_in")
                nc.sync.dma_start(ag_in0.rearrange("(k p) c -> p k c", p=128), zer[:])
                nc.gpsimd.collective_compute(
                    "AllGather", mybir.AluOpType.bypass, replica_groups=groups,
                    ins=[ag_in0.opt()], outs=[ag_out[:].opt()],
                )

                def new_xh():
                    t = spool.tile([128, 2 * HC, COLS], BF, tag="xh", bufs=3, name="xh")
                    nc.sync.dma_start(
                        t[:], ag_out[0:2].rearrange("s (k p) c -> p (s k) c", p=128)
                    )
                    return t

                def new_emb(k):
                    t = spool.tile([128, HC, COLS], BF, tag="emb", bufs=3, name="emb")
                    nc.sync.dma_start(
                        t[:], emb_stream[k].rearrange("(k p) c -> p k c", p=128)
                    )
                    return t

                def ih_group(ihdst, m, emb_t, xh_t, half):
                    # ih for gate chunk m over one half-chunk of columns
                    c0, c1 = half * HCOLS, (half + 1) * HCOLS
                    ps = pih.tile([128, HCOLS], F32, tag="ihps", name="ihps")
                    for kk in range(HC):
                        nc.tensor.matmul(
                            ps[:], wemb_sb[:, kk, m * 128:(m + 1) * 128],
                            emb_t[:, kk, c0:c1], start=(kk == 0), stop=False,
                        )
                    for kk in range(2 * HC):
                        nc.tensor.matmul(
                            ps[:], win_sb[:, kk, m * 128:(m + 1) * 128],
                            xh_t[:, kk, c0:c1], start=False, stop=(kk == 2 * HC - 1),
                        )
                    nc.scalar.add(ihdst[:, m, :], ps[:], biasp_sb[:, m:m + 1])

                def new_ih(tag):
                    return spool.tile([128, MC, HCOLS], BF, tag=tag, bufs=2, name=tag)

                # prologue: emb/xh and full ih for iteration 0
                emb_cur = new_emb(0)
                xh_cur = new_xh()
                ihA_cur = new_ih("ihA")
                ihB_cur = new_ih("ihB")
                for m in range(MC):
                    ih_group(ihA_cur, m, emb_cur, xh_cur, 0)
                for m in range(MC):
                    ih_group(ihB_cur, m, emb_cur, xh_cur, 1)

                accum_prev = None
                c_cur = None
                for k in range(iters):
                    last = k == iters - 1
                    if not last:
                        emb_next = new_emb(k + 1)
                        xh_next = new_xh()
                        ihA_next = new_ih("ihA")
                        ihB_next = new_ih("ihB")

                    # dec extraction for chunk k-3 (reads AG(k-1) output;
                    # emitted before this iteration's AG write -> safe WAR)
                    if k >= LAG + 1:
                        s23 = spool.tile([128, 2, HC, COLS], BF, tag="s23", bufs=2)
                        nc.sync.dma_start(
                            s23[:], ag_out[2:4].rearrange("s (k p) c -> p s k c", p=128)
                        )
                        dsum = spool.tile([128, B, HC, CH], BF, tag="dsum", bufs=2)
                        nc.gpsimd.tensor_add(
                            dsum[:],
                            s23[:, 0].rearrange("p k (s b) -> p b k s", b=B),
                            s23[:, 1].rearrange("p k (s b) -> p b k s", b=B),
                        )
                        ck = k - LAG - 1
                        nc.sync.dma_start(
                            dec2[:, :, ck * HC * CH:(ck + 1) * HC * CH].rearrange(
                                "b p x -> p b x"
                            ),
                            dsum.rearrange("p b k s -> p b (k s)"),
                        )

                    # ---------- state carry / blend ----------
                    accum = spool.tile([128, HC, (CH + 1) * B], BF, tag="accum")
                    if k == 0:
                        nc.vector.tensor_copy(accum[:, :, 0:B], hinit_sb[:])
                        c0t = steppool.tile([128, HC, B], F32, tag="c")
                        nc.vector.tensor_copy(c0t[:], cinit_sb[:])
                        c_cur = c0t
                    elif k == LAG:
                        # L1 cores reset to init (alpha=0); L0 keep (alpha=1)
                        nc.vector.scalar_tensor_tensor(
                            accum[:, :, 0:B], accum_prev[:, :, CH * B:],
                            alpha_sb[:, 0:1], hinite_sb[:],
                            mybir.AluOpType.mult, mybir.AluOpType.add,
                        )
                        cbl = steppool.tile([128, HC, B], F32, tag="c")
                        nc.vector.scalar_tensor_tensor(
                            cbl[:], c_cur[:], alpha_sb[:, 0:1], cinite_sb[:],
                            mybir.AluOpType.mult, mybir.AluOpType.add,
                        )
                        c_cur = cbl
                    else:
                        nc.vector.tensor_copy(accum[:, :, 0:B], accum_prev[:, :, CH * B:])

                    # ---------- CH recurrence steps with interleaved ih fill ----------
                    for s in range(CH):
                        ihs = ihA_cur if s < CH // 2 else ihB_cur
                        scol = (s % (CH // 2)) * B
                        # one PSUM tile per gate group (g,i,f,o) so each
                        # group's chain ops depend only on its own 16 MMs and
                        # start mid-burst instead of after all 64.
                        gp = {}
                        for gi, m0 in enumerate((12, 0, 4, 8)):
                            gt = pg.tile([128, HC, B], F32, tag=f"g{gi}", name=f"g{gi}", bufs=1)
                            gp[m0] = gt
                            for m in range(m0, m0 + HC):
                                for kk in range(HC):
                                    nc.tensor.matmul(
                                        gt[:, m - m0, :],
                                        whh_sb[:, kk, m * 128:(m + 1) * 128],
                                        accum[:, kk, s * B:(s + 1) * B],
                                        start=(kk == 0), stop=(kk == HC - 1),
                                    )
                        gadd = steppool.tile([128, MC, B], F32, tag="g_sb")
                        sig = steppool.tile([128, 12, B], F32, tag="sig")
                        tg = steppool.tile([128, HC, B], F32, tag="tg")
                        # g group
                        nc.vector.tensor_tensor(
                            gadd[:, 12:16, :], gp[12][:],
                            ihs[:, 12:16, scol:scol + B], mybir.AluOpType.add,
                        )
                        nc.scalar.activation(tg[:], gadd[:, 12:16, :], Tanh)
                        # i group
                        nc.vector.tensor_tensor(
                            gadd[:, 0:4, :], gp[0][:],
                            ihs[:, 0:4, scol:scol + B], mybir.AluOpType.add,
                        )
                        nc.scalar.activation(sig[:, 0:4, :], gadd[:, 0:4, :], Sig)
                        m2 = steppool.tile([128, HC, B], F32, tag="m2")
                        nc.gpsimd.tensor_mul(m2[:], sig[:, 0:4, :], tg[:])
                        # f group
                        nc.vector.tensor_tensor(
                            gadd[:, 4:8, :], gp[4][:],
                            ihs[:, 4:8, scol:scol + B], mybir.AluOpType.add,
                        )
                        nc.scalar.activation(sig[:, 4:8, :], gadd[:, 4:8, :], Sig)
                        m1 = steppool.tile([128, HC, B], F32, tag="m1")
                        nc.vector.tensor_mul(m1[:], sig[:, 4:8, :], c_cur[:])
                        # o group
                        nc.vector.tensor_tensor(
                            gadd[:, 8:12, :], gp[8][:],
                            ihs[:, 8:12, scol:scol + B], mybir.AluOpType.add,
                        )
                        nc.scalar.activation(sig[:, 8:12, :], gadd[:, 8:12, :], Sig)
                        # state update
                        c_new = steppool.tile([128, HC, B], F32, tag="c")
                        nc.gpsimd.tensor_add(c_new[:], m1[:], m2[:])
                        tc_t = steppool.tile([128, HC, B], F32, tag="tc")
                        nc.scalar.activation(tc_t[:], c_new[:], Tanh)
                        nc.vector.tensor_mul(
                            accum[:, :, (s + 1) * B:(s + 2) * B], sig[:, 8:12, :], tc_t[:]
                        )
                        c_cur = c_new
                        # filler: ih matmuls for the next iteration, emitted
                        # last so their bias-adds queue behind the chain ops.
                        if not last:
                            if 8 <= s < 16:
                                for m in (2 * (s - 8), 2 * (s - 8) + 1):
                                    ih_group(ihB_next, m, emb_next, xh_next, 1)
                            elif s >= 16:
                                ih_group(ihA_next, s - 16, emb_next, xh_next, 0)

                    # ---------- exchange ----------
                    ag_in = dpool.tile([H, COLS], BF, tag="ag_in")
                    nc.sync.dma_start(
                        ag_in.rearrange("(k p) c -> p k c", p=128), accum[:, :, B:]
                    )
                    nc.gpsimd.collective_compute(
                        "AllGather", mybir.AluOpType.bypass, replica_groups=groups,
                        ins=[ag_in.opt()], outs=[ag_out[:].opt()],
                    )
                    accum_prev = accum
                    if not last:
                        emb_cur, xh_cur = emb_next, xh_next
                        ihA_cur, ihB_cur = ihA_next, ihB_next

                # tail: AG(iters-1) carries L1's final chunk (nch-1)
                s23 = spool.tile([128, 2, HC, COLS], BF, tag="s23", bufs=2)
                nc.sync.dma_start(
                    s23[:], ag_out[2:4].rearrange("s (k p) c -> p s k c", p=128)
                )
                dsum = spool.tile([128, B, HC, CH], BF, tag="dsum", bufs=2)
                nc.gpsimd.tensor_add(
                    dsum[:],
                    s23[:, 0].rearrange("p k (s b) -> p b k s", b=B),
                    s23[:, 1].rearrange("p k (s b) -> p b k s", b=B),
                )
                ck = nch - 1
                nc.sync.dma_start(
                    dec2[:, :, ck * HC * CH:(ck + 1) * HC * CH].rearrange(
                        "b p x -> p b x"
                    ),
                    dsum.rearrange("p b k s -> p b (k s)"),
                )

            # =================== post phase ===================
            with (
                tc.tile_pool(name="ppool", bufs=1) as ppool,
                tc.tile_pool(name="pstep", bufs=3) as pstep,
                tc.tile_pool(name="ppost", bufs=2, space="PSUM") as ppost,
            ):
                from concourse.masks import make_identity
                identity = ppool.tile([128, 128], BF, tag="ident")
                make_identity(nc, identity[:])
                ones1 = ppool.tile([1, 128], BF, tag="ones1")
                nc.vector.memset(ones1[:], 1.0)

                wattT_sb = ppool.tile([128, HC, H], BF, tag="wattT")
                nc.sync.dma_start(wattT_sb[:], w_attT.rearrange("(k p) m -> p k m", p=128))
                batt_sb = ppool.tile([128, HC], F32, tag="batt")
                nc.sync.dma_start(batt_sb[:], b_att_in[:])
                wfc_sb = ppool.tile([128, 2 * HC, OUT], BF, tag="wfc")
                nc.sync.dma_start(wfc_sb[:], w_fcT.rearrange("(k p) m -> p k m", p=128))
                bfc_sb = ppool.tile([1, OUT], BF, tag="bfc")
                nc.sync.dma_start(bfc_sb[:], b_fc_row[:])
                idx_sb = ppool.tile([128, LBC], I32, tag="idx")
                nc.sync.dma_start(idx_sb[:], dec_idx[:])
                valid_sb = ppool.tile([128, LBC * TC], F32, tag="valid")
                nc.sync.dma_start(valid_sb[:], valid_in[:])

                nhalf_cols = OUT // 2
                for j in range(LBC):
                    dec_sb = ppool.tile([128, nch * HC * CH], BF, tag="dec", bufs=2)
                    nc.gpsimd.indirect_dma_start(
                        out=dec_sb[:],
                        out_offset=None,
                        in_=dec2.rearrange("b p x -> (b p) x"),
                        in_offset=bass.IndirectOffsetOnAxis(ap=idx_sb[:, j:j + 1], axis=0),
                    )
                    # reorder to hk-major so matmul lhsT slices have one free dim
                    dec_kb = ppool.tile([128, HC, nch * CH], BF, tag="deck", bufs=2)
                    nc.gpsimd.tensor_copy(
                        dec_kb.rearrange("p k (n s) -> p k n s", s=CH),
                        dec_sb.rearrange("p (n k s) -> p k n s", k=HC, s=CH),
                    )
                    encT_sb = ppool.tile([128, HC, S], BF, tag="encT", bufs=2)
                    nc.sync.dma_start(
                        encT_sb[:], encT_rhs[j].rearrange("(k p) s -> p k s", p=128)
                    )
                    enc_sb = ppool.tile([128, SC, H], BF, tag="enc", bufs=2)
                    nc.sync.dma_start(
                        enc_sb[:], enc_lhsT[j].rearrange("(k p) h -> p k h", p=128)
                    )
                    mask_sb = ppool.tile([1, S], BF, tag="mask", bufs=2)
                    nc.sync.dma_start(mask_sb[:], mask_row[j])

                    # enc_projT [H, S]
                    epT = ppool.tile([128, HC, S], BF, tag="epT", bufs=2)
                    for m in range(HC):
                        pp = ppost.tile([128, S], F32, tag="pp")
                        for e in range(HC):
                            nc.tensor.matmul(
                                pp[:], wattT_sb[:, e, m * 128:(m + 1) * 128],
                                encT_sb[:, e, :], start=(e == 0), stop=(e == HC - 1),
                            )
                        nc.vector.tensor_scalar_add(epT[:, m, :], pp[:], batt_sb[:, m:m + 1])

                    # attention over 128-timestep blocks
                    attT = ppool.tile([128, SC, T], BF, tag="attT", bufs=2)
                    for tb in range(TC):
                        sp = ppost.tile([128, S], F32, tag="pp")
                        for hk in range(HC):
                            nc.tensor.matmul(
                                sp[:], dec_kb[:, hk, tb * 128:(tb + 1) * 128],
                                epT[:, hk, :], start=(hk == 0), stop=False,
                            )
                        nc.tensor.matmul(
                            sp[:], ones1[0:1, :], mask_sb[:], start=False, stop=True,
                        )
                        mx = pstep.tile([128, 1], F32, tag="mx")
                        nc.vector.reduce_max(mx[:], sp[:], axis=mybir.AxisListType.X)
                        negmax = pstep.tile([128, 1], F32, tag="negmax")
                        nc.scalar.mul(negmax[:], mx[:], -1.0)
                        att = pstep.tile([128, S], BF, tag="att")
                        sumexp = pstep.tile([128, 1], F32, tag="sumexp")
                        nc.scalar.activation(
                            att[:], sp[:], Exp, bias=negmax[:], accum_out=sumexp[:],
                        )
                        recip = pstep.tile([128, 1], F32, tag="recip")
                        nc.vector.reciprocal(recip[:], sumexp[:])
                        attn = pstep.tile([128, S], BF, tag="attn")
                        nc.vector.tensor_scalar_mul(attn[:], att[:], recip[:])
                        for sk in range(SC):
                            tp = ppost.tile([128, 128], BF, tag="ptr")
                            nc.tensor.transpose(
                                tp[:], attn[:, sk * 128:(sk + 1) * 128], identity[:]
                            )
                            nc.vector.tensor_copy(
                                attT[:, sk, tb * 128:(tb + 1) * 128], tp[:]
                            )

                    # context ctxT [H, T]
                    ctxT = ppool.tile([128, HC, T], BF, tag="ctxT", bufs=2)
                    for m in range(HC):
                        cp = ppost.tile([128, T], F32, tag="pp")
                        for sk in range(SC):
                            nc.tensor.matmul(
                                cp[:], enc_sb[:, sk, m * 128:(m + 1) * 128],
                                attT[:, sk, :], start=(sk == 0), stop=(sk == SC - 1),
                            )
                        nc.vector.tensor_copy(ctxT[:, m, :], cp[:])

                    # fc: [128 t, OUT] blocks
                    for tb in range(TC):
                        for nh in range(2):
                            fp = ppost.tile([128, nhalf_cols], F32, tag="fp")
                            for kk in range(2 * HC):
                                lhs = (dec_kb[:, kk, tb * 128:(tb + 1) * 128] if kk < HC
                                       else ctxT[:, kk - HC, tb * 128:(tb + 1) * 128])
                                nc.tensor.matmul(
                                    fp[:], lhs,
                                    wfc_sb[:, kk, nh * nhalf_cols:(nh + 1) * nhalf_cols],
                                    start=(kk == 0), stop=False,
                                )
                            nc.tensor.matmul(
                                fp[:], ones1[0:1, :],
                                bfc_sb[:, nh * nhalf_cols:(nh + 1) * nhalf_cols],
                                start=False, stop=True,
                            )
                            osb = pstep.tile([128, nhalf_cols], F32, tag="osb")
                            nc.vector.tensor_scalar_mul(
                                osb[:], fp[:], valid_sb[:, j * TC + tb:j * TC + tb + 1]
                            )
                            nc.sync.dma_start(
                                out_d[j, tb * 128:(tb + 1) * 128,
                                      nh * nhalf_cols:(nh + 1) * nhalf_cols],
                                osb[:],
                            )

    nc.compile()
    return nc


# ---------------- host-side preparation ----------------

def _prep_inputs(inputs, nch=NCH, fp8=False):
    iters = nch + LAG
    t_total = nch * CH
    perm = _gate_perm()

    trg = np.asarray(inputs["trg_inputs"]).astype(np.int64)
    trg_len = np.asarray(inputs["trg_len"]).astype(np.int64)
    enc = _f32(inputs["encoder_outputs"])
    h0 = _f32(inputs["h0"]).reshape(L, 2, B, H)
    c0 = _f32(inputs["c0"]).reshape(L, 2, B, H)
    embed = _f32(inputs["embed"])
    W_ih0 = _f32(inputs["W_ih0"])          # [2, 4H, E]
    W_ih1 = _f32(inputs["W_ih1"])[0]       # [2, 4H, 2H]
    W_hh = _f32(inputs["W_hh"])            # [L, 2, 4H, H]
    b_ih = _f32(inputs["b_ih"])            # [L, 2, 4H]
    b_hh = _f32(inputs["b_hh"])
    W_att = _f32(inputs["W_att"])          # [H, H]
    b_att = _f32(inputs["b_att"])          # [H]
    W_fc = _f32(inputs["W_fc"])            # [OUT, 2H]
    b_fc = _f32(inputs["b_fc"])            # [OUT]

    # embedding stream [iters, E, COLS]; emb_stream[k,e,s*B+b] = X[b,32k+s,e]
    X = embed[trg[:, :t_total]]                       # [B, t, E]
    es = np.zeros((iters, E, COLS), np.float32)
    xt = X.transpose(2, 1, 0)                         # [E, t, B]
    es[:nch] = (
        xt.reshape(E, nch, CH, B).transpose(1, 0, 2, 3).reshape(nch, E, COLS)
    )
    es = _bf(es)

    cells = [(0, 0), (0, 1), (1, 0), (1, 1)]          # (layer, dir)
    zeros_emb = _bf(np.zeros((E, 4 * H)))
    zeros_in = _bf(np.zeros((2 * H, 4 * H)))

    valid_f = (np.arange(t_total)[None, :] < trg_len[:, None]).astype(np.float32)
    mask_f = np.where(np.arange(S)[None, :] < trg_len[:, None], 0.0, -1e30).astype(np.float32)
    encT = enc.transpose(0, 2, 1)                     # [B, H, S]

    pidx = np.arange(128)

    in_maps = []
    for c in range(N_CORES):
        cell = c % 4
        layer, d = cells[cell]
        if layer == 0:
            wemb = _bf(W_ih0[d][perm].T)              # [E, 4H]
            win = zeros_in
            a = 1.0
        else:
            wemb = zeros_emb
            win = _bf(W_ih1[d][perm].T)               # [2H, 4H]
            a = 0.0
        whh_np = W_hh[layer, d][perm].T               # [H, 4H]
        whh = _fp8(whh_np) if fp8 else _bf(whh_np)
        bp = (b_ih[layer, d] + b_hh[layer, d])[perm]
        biasp = _f32(bp.reshape(MC, 128).T)           # [128, MC]
        hin = h0[layer, d]                            # [B, H]
        cin = c0[layer, d]
        h_init = hin.T.reshape(HC, 128, B).transpose(1, 0, 2)   # [128,HC,B]
        c_init = cin.T.reshape(HC, 128, B).transpose(1, 0, 2)

        # post-phase batch pair for this core
        lbs = [2 * c, 2 * c + 1]

        didx = np.stack([lbs[0] * 128 + pidx, lbs[1] * 128 + pidx], axis=1).astype(np.int32)
        vpt = np.zeros((128, LBC * TC), np.float32)
        for j in range(LBC):
            for tb in range(TC):
                vpt[:, j * TC + tb] = valid_f[lbs[j], tb * 128:(tb + 1) * 128]

        m = dict(
            w_emb=wemb, w_in=win, w_hh=whh, biasp=biasp,
            h_init=_bf(h_init), c_init=_f32(c_init),
            h_init_eff=_bf(h_init * (1.0 - a)), c_init_eff=_f32(c_init * (1.0 - a)),
            alpha=_f32(np.full((128, 1), a)),
            emb_stream=es,
            enc_lhsT=_bf(enc[lbs]),                   # [2, S, H]
            encT_rhs=_bf(encT[lbs]),                  # [2, H, S]
            w_attT=_bf(W_att.T),
            b_att_in=_f32(b_att.reshape(HC, 128).T),
            mask_row=_bf(mask_f[lbs][:, None, :]),    # [2,1,S]
            valid_in=vpt,
            w_fcT=_bf(W_fc.T),                        # [2H, OUT]
            b_fc_row=_bf(b_fc[None, :]),
            dec_idx=didx,
        )
        in_maps.append(m)
    return in_maps


_NC_CACHE = {}


def kernel(**inputs) -> np.ndarray:
    nch = int(os.environ.get("KERNEL_NCH", NCH))
    fp8 = os.environ.get("KERNEL_FP8", "0") == "1"
    key = (nch, fp8)
    if key not in _NC_CACHE:
        _NC_CACHE[key] = build_nc(nch, fp8)
    nc = _NC_CACHE[key]
    in_maps = _prep_inputs(inputs, nch, fp8)
    r = run_bass_kernel_spmd(nc, in_maps, list(range(N_CORES)))
    t_total = nch * CH
    full = np.zeros((B, t_total, OUT), np.float32)
    for c in range(N_CORES):
        o = np.asarray(r.results[c]["out"], np.float32)
        full[2 * c] = o[0]
        full[2 * c + 1] = o[1]
    return full
